# revision 4
# baseline (speedup 1.0000x reference)
"""Gumbel-softmax hard sampling (B=4096, C=32000, f32) on 8 trn2 NeuronCores.

Math: output = one_hot(argmax(softmax((logits+g)/tau))) with tau=1 and
g = -log(EPS - log(u+EPS)).  Softmax is strictly monotonic per row, so
argmax(softmax(s)) == argmax(s): we only need argmax(logits + g) and a
one-hot materialization — no exp/softmax on device.

Sharding: batch rows split 8 ways (512 rows/core, pure data parallel).

Device kernel per core (512 x 32000):
  pass 1 (stream, 4 row-blocks x 8 col-tiles of [128, 4000]):
    ACT: t1 = Ln(u + EPS); t2 = Ln(EPS - t1)        (2 LUT passes)
    DVE: score = logits - t2                        (in-place over logits tile)
    DVE: per-tile top-8 values + indices (max / max_index)
    cross-tile argmax via is_equal indicator + reduce_sum (tiny ops)
  pass 2: one-hot = Relu(1 - |iota - idx|) on ACT (exact for integer-valued
    f32), streamed straight to DRAM.

Host patch: the ACT Ln LUT differs from XLA's log by ~ulps, which can flip
rows whose top-2 scores are within that noise.  The kernel also returns the
per-tile top-8 candidate indices; the host recomputes those candidates'
scores with jax f32 (bitwise-matching the reference expression) and patches
any rows where the winner differs.  Candidate sets always contain the true
argmax (per-tile top-8 containment), so the patched output is exact up to
reference-internal softmax rounding ties (P ~ 1e-4).
"""

import sys

if "/opt/trn_rl_repo" not in sys.path:
    sys.path.insert(0, "/opt/trn_rl_repo")

from contextlib import ExitStack  # noqa: E402

import numpy as np  # noqa: E402

import concourse.bass as bass  # noqa: E402
import concourse.tile as tile  # noqa: E402
from concourse import bacc, mybir  # noqa: E402
from concourse.bass_utils import run_bass_kernel_spmd  # noqa: E402

EPS = 1e-10
B, C = 4096, 32000
N_CORES = 8
ROWS = B // N_CORES          # 512 rows per core
P = 128                      # partitions per row-block
N_BLOCKS = ROWS // P         # 4
W = 4000                     # col-tile width
T = C // W                   # 8 col-tiles
F32 = mybir.dt.float32
U32 = mybir.dt.uint32
I32 = mybir.dt.int32


def build_program():
    nc = bacc.Bacc(
        "TRN2", target_bir_lowering=False, debug=False, num_devices=N_CORES
    )
    # EPS as a per-partition const AP for activation bias
    eps_t = nc.alloc_sbuf_tensor("const-eps", [128, 1], F32)
    nc.gpsimd.memset(eps_t.ap(), EPS)
    nc.const_aps.aps[(F32, EPS)] = eps_t.ap()
    nc.all_engine_barrier()
    logits = nc.dram_tensor("logits", [ROWS, C], F32, kind="ExternalInput").ap()
    u = nc.dram_tensor("u", [ROWS, C], F32, kind="ExternalInput").ap()
    onehot = nc.dram_tensor("onehot", [ROWS, C], F32, kind="ExternalOutput").ap()
    chosen_d = nc.dram_tensor("chosen", [ROWS], F32, kind="ExternalOutput").ap()
    cand_d = nc.dram_tensor("cand", [ROWS, T * 8], F32, kind="ExternalOutput").ap()

    with tile.TileContext(nc) as tc, ExitStack() as ctx:
        lpool = ctx.enter_context(tc.tile_pool(name="lpool", bufs=3))
        upool = ctx.enter_context(tc.tile_pool(name="upool", bufs=3))
        ohpool = ctx.enter_context(tc.tile_pool(name="ohpool", bufs=3))
        cpool = ctx.enter_context(tc.tile_pool(name="cpool", bufs=1))
        spool = ctx.enter_context(tc.tile_pool(name="spool", bufs=2))

        # iota row 0..W-1 (same in every partition), converted to f32 once
        iota_i = cpool.tile([P, W], I32)
        nc.gpsimd.iota(iota_i[:], pattern=[[1, W]], base=0, channel_multiplier=0)
        iota_f = cpool.tile([P, W], F32)
        nc.vector.tensor_copy(iota_f[:], iota_i[:])

        for b in range(N_BLOCKS):
            r0 = b * P
            mx = spool.tile([P, T * 8], F32, tag="mx")
            mi = spool.tile([P, T * 8], U32, tag="mi")
            score_tiles = []
            for t in range(T):
                c0 = t * W
                lt = lpool.tile([P, W], F32)
                nc.sync.dma_start(lt[:], logits[r0 : r0 + P, c0 : c0 + W])
                ut = upool.tile([P, W], F32)
                nc.sync.dma_start(ut[:], u[r0 : r0 + P, c0 : c0 + W])
                # t1 = ln(u + eps); t2 = ln(eps - t1)   (in-place on ut)
                nc.scalar.activation(
                    ut[:], ut[:], mybir.ActivationFunctionType.Ln, bias=EPS, scale=1.0
                )
                nc.scalar.activation(
                    ut[:], ut[:], mybir.ActivationFunctionType.Ln, bias=EPS, scale=-1.0
                )
                # score = logits - t2   (in-place on lt)
                nc.vector.tensor_sub(lt[:], lt[:], ut[:])
                nc.vector.max(mx[:, 8 * t : 8 * t + 8], lt[:])
                nc.vector.max_index(
                    mi[:, 8 * t : 8 * t + 8], mx[:, 8 * t : 8 * t + 8], lt[:]
                )
                score_tiles.append(lt)

            # global candidate indices (f32): gif = mi + 4000*t per group
            gif = spool.tile([P, T * 8], F32, tag="gif")
            nc.vector.tensor_copy(gif[:], mi[:])
            for t in range(1, T):
                nc.vector.tensor_scalar_add(
                    gif[:, 8 * t : 8 * t + 8], gif[:, 8 * t : 8 * t + 8], float(W * t)
                )
            # cross-tile argmax: chosen = sum(gif * (mx == max(mx)))
            gmax8 = spool.tile([P, 8], F32, tag="gmax8")
            nc.vector.max(gmax8[:], mx[:])
            eq = spool.tile([P, T * 8], F32, tag="eq")
            nc.vector.tensor_scalar(
                eq[:], mx[:], gmax8[:, 0:1], None, op0=mybir.AluOpType.is_equal
            )
            nc.vector.tensor_mul(eq[:], eq[:], gif[:])
            chosen = spool.tile([P, 1], F32, tag="chosen")
            nc.vector.reduce_sum(chosen[:], eq[:], axis=mybir.AxisListType.X)

            # per-tile bias for pass 2: negadj[:, t] = -(chosen - W*t)
            negadj = spool.tile([P, T], F32, tag="negadj")
            for t in range(T):
                nc.vector.tensor_scalar(
                    negadj[:, t : t + 1],
                    chosen[:],
                    -1.0,
                    float(W * t),
                    op0=mybir.AluOpType.mult,
                    op1=mybir.AluOpType.add,
                )

            nc.sync.dma_start(chosen_d[r0 : r0 + P], chosen[:])
            nc.sync.dma_start(cand_d[r0 : r0 + P, :], gif[:])

            # pass 2: one_hot = relu(1 - |iota - adj|)
            for t in range(T):
                c0 = t * W
                oh = ohpool.tile([P, W], F32)
                nc.scalar.activation(
                    oh[:],
                    iota_f[:],
                    mybir.ActivationFunctionType.Abs,
                    bias=negadj[:, t : t + 1],
                    scale=1.0,
                )
                nc.scalar.activation(
                    oh[:], oh[:], mybir.ActivationFunctionType.Relu, bias=1.0, scale=-1.0
                )
                nc.sync.dma_start(onehot[r0 : r0 + P, c0 : c0 + W], oh[:])

    nc.compile()
    return nc


_NC_CACHE = None


def _get_program():
    global _NC_CACHE
    if _NC_CACHE is None:
        _NC_CACHE = build_program()
    return _NC_CACHE


def _host_refine(logits, u, cand_idx):
    """Recompute candidate scores with jax f32 (matches reference bitwise);
    return the reference-semantics argmax per row (first occurrence)."""
    import jax
    import jax.numpy as jnp

    rows = np.arange(B)[:, None]
    lg = logits[rows, cand_idx]
    ug = u[rows, cand_idx]
    with jax.default_device(jax.local_devices(backend="cpu")[0]):
        g = -jnp.log(EPS - jnp.log(jnp.asarray(ug) + EPS))
        sc = np.asarray(jnp.asarray(lg) + g)
    m = sc.max(axis=1, keepdims=True)
    masked = np.where(sc == m, cand_idx, np.iinfo(np.int64).max)
    return masked.min(axis=1)


def kernel(logits: np.ndarray, u: np.ndarray, **_) -> np.ndarray:
    logits = np.ascontiguousarray(logits, dtype=np.float32)
    u = np.ascontiguousarray(u, dtype=np.float32)
    nc = _get_program()
    in_maps = [
        {
            "logits": logits[i * ROWS : (i + 1) * ROWS],
            "u": u[i * ROWS : (i + 1) * ROWS],
        }
        for i in range(N_CORES)
    ]
    res = run_bass_kernel_spmd(nc, in_maps, core_ids=list(range(N_CORES)))
    out = np.concatenate([r["onehot"] for r in res.results], axis=0)
    chosen = np.concatenate([r["chosen"] for r in res.results]).reshape(B)
    cand = np.concatenate([r["cand"] for r in res.results]).reshape(B, T * 8)

    chosen_i = np.rint(chosen).astype(np.int64)
    cand_i = np.clip(np.rint(cand).astype(np.int64), 0, C - 1)
    host_idx = _host_refine(logits, u, cand_i)

    bad = np.nonzero(host_idx != chosen_i)[0]
    for r in bad:
        ci = chosen_i[r]
        if 0 <= ci < C:
            out[r, ci] = 0.0
        out[r, host_idx[r]] = 1.0
    return out


# revision 9
# speedup vs baseline: 7.4032x; 7.4032x over previous
"""Gumbel-softmax hard sampling (B=4096, C=32000, f32) on 8 trn2 NeuronCores.

Math: output = one_hot(argmax(softmax((logits+g)/tau))) with tau=1 and
g = -log(EPS - log(u+EPS)).  Softmax is strictly monotonic per row, so
argmax(softmax(s)) == argmax(s): we only need argmax(logits + g) and a
one-hot materialization — no exp/softmax on device.

Sharding: batch rows split 8 ways (512 rows/core, pure data parallel).

Device kernel per core (512 x 32000):
  pass 1 (stream, 4 row-blocks x 8 col-tiles of [128, 4000]):
    ACT: t1 = Ln(u + EPS); t2 = Ln(EPS - t1)        (2 LUT passes)
    DVE: score = logits - t2                        (in-place over logits tile)
    DVE: per-tile top-8 values + indices (max / max_index)
    cross-tile argmax via is_equal indicator + reduce_sum (tiny ops)
  pass 2: one-hot = Relu(1 - |iota - idx|) on ACT (exact for integer-valued
    f32), streamed straight to DRAM.

Host patch: the ACT Ln LUT differs from XLA's log by ~ulps, which can flip
rows whose top-2 scores are within that noise.  The kernel also returns the
per-tile top-8 candidate indices; the host recomputes those candidates'
scores with jax f32 (bitwise-matching the reference expression) and patches
any rows where the winner differs.  Candidate sets always contain the true
argmax (per-tile top-8 containment), so the patched output is exact up to
reference-internal softmax rounding ties (P ~ 1e-4).
"""

import sys

if "/opt/trn_rl_repo" not in sys.path:
    sys.path.insert(0, "/opt/trn_rl_repo")

from contextlib import ExitStack  # noqa: E402

import numpy as np  # noqa: E402

import concourse.bass as bass  # noqa: E402
import concourse.tile as tile  # noqa: E402
from concourse import bacc, mybir  # noqa: E402
from concourse.bass_utils import run_bass_kernel_spmd  # noqa: E402

EPS = 1e-10
B, C = 4096, 32000
N_CORES = 8
ROWS = B // N_CORES          # 512 rows per core
P = 128                      # partitions per row-block
N_BLOCKS = ROWS // P         # 4
W = 4000                     # col-tile width
T = C // W                   # 8 col-tiles
F32 = mybir.dt.float32
U32 = mybir.dt.uint32
I32 = mybir.dt.int32


def _setup(nc, load_eng, store_eng):
    # EPS as a per-partition const AP for activation bias
    eps_t = nc.alloc_sbuf_tensor("const-eps", [128, 1], F32)
    nc.gpsimd.memset(eps_t.ap(), EPS)
    nc.const_aps.aps[(F32, EPS)] = eps_t.ap()
    nc.all_engine_barrier()
    aps = dict(
        logits=nc.dram_tensor("logits", [ROWS, C], F32, kind="ExternalInput").ap(),
        u=nc.dram_tensor("u", [ROWS, C], F32, kind="ExternalInput").ap(),
        onehot=nc.dram_tensor("onehot", [ROWS, C], F32, kind="ExternalOutput").ap(),
        chosen=nc.dram_tensor("chosen", [ROWS], F32, kind="ExternalOutput").ap(),
        cand=nc.dram_tensor("cand", [ROWS, T * 8], F32, kind="ExternalOutput").ap(),
    )
    return aps


def _emit_blocks(nc, tc, ctx, aps, block_list, load_eng, store_eng):
    logits, u = aps["logits"], aps["u"]
    onehot, chosen_d, cand_d = aps["onehot"], aps["chosen"], aps["cand"]
    lpool = ctx.enter_context(tc.tile_pool(name="lpool", bufs=3))
    upool = ctx.enter_context(tc.tile_pool(name="upool", bufs=3))
    ohpool = ctx.enter_context(tc.tile_pool(name="ohpool", bufs=3))
    cpool = ctx.enter_context(tc.tile_pool(name="cpool", bufs=1))
    spool = ctx.enter_context(tc.tile_pool(name="spool", bufs=2))

    ld = getattr(nc, load_eng)
    st = getattr(nc, store_eng)

    # iota row 0..W-1 (same in every partition), converted to f32 once
    iota_i = cpool.tile([P, W], I32)
    nc.gpsimd.iota(iota_i[:], pattern=[[1, W]], base=0, channel_multiplier=0)
    iota_f = cpool.tile([P, W], F32)
    nc.vector.tensor_copy(iota_f[:], iota_i[:])

    def one_rep():
        for b in block_list:
            emit_block(b)

    def emit_block(b):
            r0 = b * P
            mx = spool.tile([P, T * 8], F32, tag="mx")
            mi = spool.tile([P, T * 8], U32, tag="mi")
            score_tiles = []
            for t in range(T):
                c0 = t * W
                lt = lpool.tile([P, W], F32)
                ld.dma_start(lt[:], logits[r0 : r0 + P, c0 : c0 + W])
                ut = upool.tile([P, W], F32)
                ld.dma_start(ut[:], u[r0 : r0 + P, c0 : c0 + W])
                # t1 = ln(u + eps); t2 = ln(eps - t1)   (in-place on ut)
                nc.scalar.activation(
                    ut[:], ut[:], mybir.ActivationFunctionType.Ln, bias=EPS, scale=1.0
                )
                nc.scalar.activation(
                    ut[:], ut[:], mybir.ActivationFunctionType.Ln, bias=EPS, scale=-1.0
                )
                # score = logits - t2   (in-place on lt)
                nc.vector.tensor_sub(lt[:], lt[:], ut[:])
                nc.vector.max(mx[:, 8 * t : 8 * t + 8], lt[:])
                nc.vector.max_index(
                    mi[:, 8 * t : 8 * t + 8], mx[:, 8 * t : 8 * t + 8], lt[:]
                )
                score_tiles.append(lt)

            # global candidate indices (f32): gif = mi + 4000*t per group
            gif = spool.tile([P, T * 8], F32, tag="gif")
            nc.vector.tensor_copy(gif[:], mi[:])
            for t in range(1, T):
                nc.vector.tensor_scalar_add(
                    gif[:, 8 * t : 8 * t + 8], gif[:, 8 * t : 8 * t + 8], float(W * t)
                )
            # cross-tile argmax: chosen = sum(gif * (mx == max(mx)))
            gmax8 = spool.tile([P, 8], F32, tag="gmax8")
            nc.vector.max(gmax8[:], mx[:])
            eq = spool.tile([P, T * 8], F32, tag="eq")
            nc.vector.tensor_scalar(
                eq[:], mx[:], gmax8[:, 0:1], None, op0=mybir.AluOpType.is_equal
            )
            nc.vector.tensor_mul(eq[:], eq[:], gif[:])
            chosen = spool.tile([P, 1], F32, tag="chosen")
            nc.vector.reduce_sum(chosen[:], eq[:], axis=mybir.AxisListType.X)

            # per-tile bias for pass 2: negadj[:, t] = -(chosen - W*t)
            negadj = spool.tile([P, T], F32, tag="negadj")
            for t in range(T):
                nc.vector.tensor_scalar(
                    negadj[:, t : t + 1],
                    chosen[:],
                    -1.0,
                    float(W * t),
                    op0=mybir.AluOpType.mult,
                    op1=mybir.AluOpType.add,
                )

            st.dma_start(chosen_d[r0 : r0 + P], chosen[:])
            st.dma_start(cand_d[r0 : r0 + P, :], gif[:])

            # pass 2: one_hot = relu(1 - |iota - adj|)
            for t in range(T):
                c0 = t * W
                oh = ohpool.tile([P, W], F32)
                nc.scalar.activation(
                    oh[:],
                    iota_f[:],
                    mybir.ActivationFunctionType.Abs,
                    bias=negadj[:, t : t + 1],
                    scale=1.0,
                )
                nc.scalar.activation(
                    oh[:], oh[:], mybir.ActivationFunctionType.Relu, bias=1.0, scale=-1.0
                )
                st.dma_start(onehot[r0 : r0 + P, c0 : c0 + W], oh[:])

    return one_rep


def build_program(reps: int = 1, store_eng: str = "sync", load_eng: str = "sync"):
    nc = bacc.Bacc(
        "TRN2", target_bir_lowering=False, debug=False, num_devices=N_CORES
    )
    aps = _setup(nc, load_eng, store_eng)
    with tile.TileContext(nc) as tc, ExitStack() as ctx:
        one_rep = _emit_blocks(
            nc, tc, ctx, aps, list(range(N_BLOCKS)), load_eng, store_eng
        )
        for _ in range(reps):
            one_rep()
    nc.compile()
    return nc


def build_loop_program(
    loops: int = 64, store_eng: str = "sync", load_eng: str = "sync"
):
    """Benchmark-only variant: device-side For_i loop around the full body."""
    nc = bacc.Bacc(
        "TRN2", target_bir_lowering=False, debug=False, num_devices=N_CORES
    )
    aps = _setup(nc, load_eng, store_eng)
    with tile.TileContext(nc) as tc, ExitStack() as ctx:
        one_rep = _emit_blocks(
            nc, tc, ctx, aps, list(range(N_BLOCKS)), load_eng, store_eng
        )
        with tc.For_i(0, loops, 1):
            one_rep()
    nc.compile()
    return nc


_NC_CACHE = None


def _get_program():
    global _NC_CACHE
    if _NC_CACHE is None:
        _NC_CACHE = build_program()
    return _NC_CACHE


def _host_refine(logits, u, cand_idx):
    """Recompute candidate scores with jax f32 (matches reference bitwise);
    return the reference-semantics argmax per row (first occurrence)."""
    import jax
    import jax.numpy as jnp

    rows = np.arange(B)[:, None]
    lg = logits[rows, cand_idx]
    ug = u[rows, cand_idx]
    with jax.default_device(jax.local_devices(backend="cpu")[0]):
        g = -jnp.log(EPS - jnp.log(jnp.asarray(ug) + EPS))
        sc = np.asarray(jnp.asarray(lg) + g)
    m = sc.max(axis=1, keepdims=True)
    masked = np.where(sc == m, cand_idx, np.iinfo(np.int64).max)
    return masked.min(axis=1)


def kernel(logits: np.ndarray, u: np.ndarray, **_) -> np.ndarray:
    logits = np.ascontiguousarray(logits, dtype=np.float32)
    u = np.ascontiguousarray(u, dtype=np.float32)
    nc = _get_program()
    in_maps = [
        {
            "logits": logits[i * ROWS : (i + 1) * ROWS],
            "u": u[i * ROWS : (i + 1) * ROWS],
        }
        for i in range(N_CORES)
    ]
    res = run_bass_kernel_spmd(nc, in_maps, core_ids=list(range(N_CORES)))
    out = np.concatenate([r["onehot"] for r in res.results], axis=0)
    chosen = np.concatenate([r["chosen"] for r in res.results]).reshape(B)
    cand = np.concatenate([r["cand"] for r in res.results]).reshape(B, T * 8)

    chosen_i = np.rint(chosen).astype(np.int64)
    cand_i = np.clip(np.rint(cand).astype(np.int64), 0, C - 1)
    host_idx = _host_refine(logits, u, cand_i)

    bad = np.nonzero(host_idx != chosen_i)[0]
    for r in bad:
        ci = chosen_i[r]
        if 0 <= ci < C:
            out[r, ci] = 0.0
        out[r, host_idx[r]] = 1.0
    return out
